# revision 4
# baseline (speedup 1.0000x reference)
"""Trainium2 Bass kernel for the ClipMatcher detection-loss module.

Data-parallel over B=240 frames across 8 NeuronCores (30 frames/core).
Per core: 1x1-conv head stack as float32r matmuls (channel-major),
PE-transpose of head outputs to (bl, anchor)-major, then the anchor
matching / GIoU / BCE loss reduced on-chip to 6 partial sums.
Host combines the 8 partial-sum vectors into the scalar loss.
"""
import sys

sys.path.insert(0, "/opt/trn_rl_repo")
import numpy as np

N_CORES = 8
B, C, L = 240, 256, 256
BLOC = B // N_CORES           # 30 frames per core
BL = BLOC * L                 # 7680 bl-columns per core
NT = BL // 128                # 60 transposed tiles per core
NCH = 15                      # conv chunks per core
CW = BL // NCH                # 512 columns per chunk
FPC = CW // L                 # 2 frames per chunk
LG = 12                       # tiles per loss group
NLG = NT // LG                # 5 loss groups
EPS = 1e-7
IMG = 448.0
NREG = 16
POS_THR = 0.2

_CACHE = {}


# ---------------------------------------------------------------- host math
def _make_anchors_np():
    # verbatim port of reference.make_anchors (float64 centers, float32 wh,
    # single final round to float32 — matches jax bit-for-bit)
    base = np.array([[16.0, 16.0], [32.0, 32.0], [64.0, 64.0], [128.0, 128.0]], np.float32)
    ar = np.array([0.5, 1.0, 2.0], np.float32)
    c = (np.arange(NREG) + 0.5) * IMG / NREG
    cy, cx = np.meshgrid(c, c, indexing="ij")
    centers = np.stack([cx.ravel(), cy.ravel()], -1)
    s = np.sqrt(ar)
    w = base[:, 0:1] / s[None, :]
    h = base[:, 1:2] * s[None, :]
    wh = np.stack([w, h], -1).reshape(-1, 2)
    xy = np.repeat(centers, 12, 0)
    wh = np.tile(wh, (NREG * NREG, 1))
    a = np.concatenate([xy, wh], -1) / IMG
    return a.astype(np.float32)                     # [3072, 4] xyhw


def _xyxy(b):
    cx, cy, w, h = b[..., 0], b[..., 1], b[..., 2], b[..., 3]
    return np.stack([cx - w / 2, cy - h / 2, cx + w / 2, cy + h / 2], -1)


def _np32(x):
    return np.ascontiguousarray(np.asarray(x), dtype=np.float32)


# ---------------------------------------------------------------- device program
def _build_nc():
    import concourse.mybir as mybir
    from concourse import bacc, tile
    from contextlib import ExitStack

    f32 = mybir.dt.float32
    f32r = mybir.dt.float32r
    AL = mybir.AluOpType
    AF = mybir.ActivationFunctionType

    nc = bacc.Bacc("TRN2", target_bir_lowering=False, debug=False, num_devices=N_CORES)

    feat_d = nc.dram_tensor("feat", [BLOC, C, L], f32, kind="ExternalInput")
    w0_d = nc.dram_tensor("w0", [2, 128, 512], f32, kind="ExternalInput")
    wr_d = nc.dram_tensor("wr", [3, 2, 128, 256], f32, kind="ExternalInput")
    wc_d = nc.dram_tensor("wc", [3, 2, 128, 256], f32, kind="ExternalInput")
    wh_d = nc.dram_tensor("wh", [4, 128, 60], f32, kind="ExternalInput")
    sbv_d = nc.dram_tensor("sbv", [128, 34], f32, kind="ExternalInput")
    idn_d = nc.dram_tensor("idn", [128, 128], f32, kind="ExternalInput")
    anc_d = nc.dram_tensor("anc", [128, NT, 48], f32, kind="ExternalInput")
    ancx_d = nc.dram_tensor("ancx", [128, 4, NT, 12], f32, kind="ExternalInput")
    gt48_d = nc.dram_tensor("gt48", [128, NT, 48], f32, kind="ExternalInput")
    gtx_d = nc.dram_tensor("gtx", [128, 6, NT, 12], f32, kind="ExternalInput")
    out_d = nc.dram_tensor("o", [1, 6], f32, kind="ExternalOutput")

    V = nc.vector
    S = nc.scalar

    with tile.TileContext(nc) as tc, ExitStack() as ctx:
        cp = ctx.enter_context(tc.tile_pool(name="const", bufs=1))
        xp = ctx.enter_context(tc.tile_pool(name="xin", bufs=4))
        ap = ctx.enter_context(tc.tile_pool(name="acts", bufs=6))
        hp = ctx.enter_context(tc.tile_pool(name="hd", bufs=2))
        pp = ctx.enter_context(tc.tile_pool(name="ps", bufs=4, space="PSUM"))
        tp = ctx.enter_context(tc.tile_pool(name="pst", bufs=2, space="PSUM"))
        sp = ctx.enter_context(tc.tile_pool(name="scr", bufs=14))
        sp6 = ctx.enter_context(tc.tile_pool(name="scr6", bufs=4))

        # ---- persistent loads
        w0s = [cp.tile([128, 512], f32r, tag=f"w0_{k}", name=f"w0_{k}") for k in range(2)]
        wrs = [[cp.tile([128, 256], f32r, tag=f"wr_{l}_{k}", name=f"wr_{l}_{k}") for k in range(2)] for l in range(3)]
        wcs = [[cp.tile([128, 256], f32r, tag=f"wc_{l}_{k}", name=f"wc_{l}_{k}") for k in range(2)] for l in range(3)]
        whs = [cp.tile([128, 60], f32r, tag=f"wh_{k}", name=f"wh_{k}") for k in range(4)]
        for k in range(4):
            nc.gpsimd.dma_start(whs[k][:], wh_d.ap()[k])
        for k in range(2):
            nc.gpsimd.dma_start(w0s[k][:], w0_d.ap()[k])
            for l in range(3):
                nc.gpsimd.dma_start(wrs[l][k][:], wr_d.ap()[l, k])
                nc.gpsimd.dma_start(wcs[l][k][:], wc_d.ap()[l, k])
        sbv = cp.tile([128, 34], f32, tag="sbv")
        nc.sync.dma_start(sbv[:], sbv_d.ap())
        idn = cp.tile([128, 128], f32, tag="idn")
        nc.sync.dma_start(idn[:], idn_d.ap())
        anc = cp.tile([128, NT * 48], f32, tag="anc")
        nc.sync.dma_start(anc[:], anc_d.ap().rearrange("p t c -> p (t c)"))
        ancx = cp.tile([128, 4 * NT * 12], f32, tag="ancx")
        nc.sync.dma_start(ancx[:], ancx_d.ap().rearrange("p k t j -> p (k t j)"))
        gt48 = cp.tile([128, NT * 48], f32, tag="gt48")
        nc.sync.dma_start(gt48[:], gt48_d.ap().rearrange("p t c -> p (t c)"))
        gtx = cp.tile([128, 6 * NT * 12], f32, tag="gtx")
        nc.sync.dma_start(gtx[:], gtx_d.ap().rearrange("p k t j -> p (k t j)"))

        outT = cp.tile([128, NT * 60], f32, tag="outT")
        acc = cp.tile([128, 6], f32, tag="acc")
        V.memset(acc[:], 0.0)

        # scale/shift column map in sbv
        SC_L0 = 0            # 4 cols
        SC_R = lambda l: 4 + 2 * l
        SC_C = lambda l: 10 + 2 * l
        TOF = 16             # shift block offset
        BH_R, BH_C = 32, 33  # head biases

        # ---- conv + head + transpose, chunked over bl
        for g in range(NCH):
            xk = []
            for k in range(2):
                xt = xp.tile([128, FPC, L], f32r, tag="xt")
                src = feat_d.ap()[FPC * g:FPC * (g + 1), 128 * k:128 * (k + 1), :]
                nc.gpsimd.dma_start(xt[:], src.rearrange("f c l -> c f l"))
                xk.append(xt)

            # L0: 256 -> 512
            y0 = []
            for m in range(4):
                ps = pp.tile([128, 512], f32, tag="ps")
                nc.tensor.matmul(ps[:], w0s[0][:, 128 * m:128 * (m + 1)], xk[0][:], start=True, stop=False)
                nc.tensor.matmul(ps[:], w0s[1][:, 128 * m:128 * (m + 1)], xk[1][:], start=False, stop=True)
                y = ap.tile([128, 512], f32r, tag="y0")
                S.activation(y[:], ps[:], AF.Prelu,
                             bias=sbv[:, TOF + SC_L0 + m:TOF + SC_L0 + m + 1],
                             scale=sbv[:, SC_L0 + m:SC_L0 + m + 1], alpha=0.01)
                y0.append(y)

            # three 256->256 convs on each branch
            branches = []
            for bi, (wts, sc) in enumerate([(wrs, SC_R), (wcs, SC_C)]):
                cur = [y0[0 + 2 * bi], y0[1 + 2 * bi]]
                for l in range(3):
                    nxt = []
                    for m in range(2):
                        ps = pp.tile([128, 512], f32, tag="ps")
                        nc.tensor.matmul(ps[:], wts[l][0][:, 128 * m:128 * (m + 1)], cur[0][:], start=True, stop=False)
                        nc.tensor.matmul(ps[:], wts[l][1][:, 128 * m:128 * (m + 1)], cur[1][:], start=False, stop=True)
                        y = ap.tile([128, 512], f32r, tag=f"b{bi}l{l}")
                        col = sc(l) + m
                        S.activation(y[:], ps[:], AF.Prelu,
                                     bias=sbv[:, TOF + col:TOF + col + 1],
                                     scale=sbv[:, col:col + 1], alpha=0.01)
                        nxt.append(y)
                    cur = nxt
                branches.append(cur)
            regf, clsf = branches

            # heads: block-diagonal [512, 60] weight over (reg ⊕ cls) features
            psh = pp.tile([128, 512], f32, tag="ps")
            feats4 = [regf[0], regf[1], clsf[0], clsf[1]]
            for k in range(4):
                nc.tensor.matmul(psh[0:60, :], whs[k][:], feats4[k][:],
                                 start=(k == 0), stop=(k == 3))
            hd = hp.tile([60, 512], f32, tag="hd")
            S.activation(hd[:], psh[0:60, :], AF.Identity, bias=sbv[0:60, BH_R:BH_R + 1])

            # transpose [60, 512] -> 4 x [128, 60] in one psum bank, copy to outT
            pst = tp.tile([128, 240], f32, tag="pst")
            for s4 in range(4):
                nc.tensor.transpose(pst[:, 60 * s4:60 * (s4 + 1)],
                                    hd[:, 128 * s4:128 * (s4 + 1)], idn[0:60, 0:60])
            V.tensor_copy(outT[:, 240 * g:240 * (g + 1)], pst[:])

        # ---- loss phase over transposed head outputs
        ot3 = outT[:].rearrange("p (t ch) -> p t ch", ch=60)
        anc3 = anc[:].rearrange("p (t c) -> p t c", c=48)
        gt483 = gt48[:].rearrange("p (t c) -> p t c", c=48)
        ancx4 = ancx[:].rearrange("p (k t j) -> p k t j", k=4, t=NT)
        gtx4 = gtx[:].rearrange("p (k t j) -> p k t j", k=6, t=NT)

        W12 = LG * 12   # 144
        W48 = LG * 48   # 576

        scounter = [0]

        def s12():
            scounter[0] += 1
            return sp.tile([128, W12], f32, tag="s12", name=f"s12_{scounter[0]}")

        def s48():
            scounter[0] += 1
            return sp6.tile([128, W48], f32, tag="s48", name=f"s48_{scounter[0]}")

        def tt(out, a, b, op):
            V.tensor_tensor(out, a, b, op)

        for gi in range(NLG):
            tsl = slice(LG * gi, LG * (gi + 1))
            Rv = ot3[:, tsl, 0:48]
            Zv = ot3[:, tsl, 48:60]
            AX1, AY1, AX2, AY2 = (ancx4[:, k, tsl, :] for k in range(4))
            GX1, GY1, GX2, GY2, GAR, SAB = (gtx4[:, k, tsl, :] for k in range(6))

            accg = sp.tile([128, 6], f32, tag="accg")

            # pred boxes (xyhw) and |pred - gt| summed over the 4 coords
            pred = s48()
            tt(pred[:].rearrange("p (t c) -> p t c", c=48), Rv, anc3[:, tsl, :], AL.add)
            d48 = s48()
            tt(d48[:], pred[:], gt48[:, W48 * gi:W48 * (gi + 1)], AL.subtract)
            lsum = s12()
            V.tensor_reduce(lsum[:].rearrange("p (t j) -> p t j", j=12),
                            d48[:].rearrange("p (t j c) -> p t j c", j=12, c=4),
                            mybir.AxisListType.X, AL.add, apply_absolute_value=True)

            # pos = IoU(anchor, gt) > 0.2  (exact f32 chain, ref op order)
            ix1, iy1, ix2, iy2 = s12(), s12(), s12(), s12()
            tt(ix1[:].rearrange("p (t j) -> p t j", j=12), AX1, GX1, AL.max)
            tt(iy1[:].rearrange("p (t j) -> p t j", j=12), AY1, GY1, AL.max)
            tt(ix2[:].rearrange("p (t j) -> p t j", j=12), AX2, GX2, AL.min)
            tt(iy2[:].rearrange("p (t j) -> p t j", j=12), AY2, GY2, AL.min)
            iw, ih = s12(), s12()
            tt(iw[:], ix2[:], ix1[:], AL.subtract)
            V.tensor_scalar_max(iw[:], iw[:], 0.0)
            tt(ih[:], iy2[:], iy1[:], AL.subtract)
            V.tensor_scalar_max(ih[:], ih[:], 0.0)
            ia = s12()
            tt(ia[:], iw[:], ih[:], AL.mult)
            iu = s12()
            tt(iu[:].rearrange("p (t j) -> p t j", j=12),
               SAB, ia[:].rearrange("p (t j) -> p t j", j=12), AL.subtract)
            V.tensor_scalar_add(iu[:], iu[:], EPS)
            V.reciprocal(iu[:], iu[:])
            iou = s12()
            tt(iou[:], ia[:], iu[:], AL.mult)
            pos = s12()
            V.tensor_scalar(pos[:], iou[:], POS_THR, None, AL.is_gt)
            V.reduce_sum(accg[:, 0:1], pos[:], mybir.AxisListType.X)

            # center+hw term: sum(pos * lsum)
            junk = s12()
            V.scalar_tensor_tensor(junk[:], lsum[:], 1.0, pos[:], AL.mult, AL.mult,
                                   accum_out=accg[:, 1:2])

            # pred xyxy
            p4 = pred[:].rearrange("p (t j c) -> p t j c", j=12, c=4)
            px1, py1, px2, py2 = s12(), s12(), s12(), s12()
            for dst, cc, cw in ((px1, 0, 2), (py1, 1, 3)):
                V.scalar_tensor_tensor(dst[:].rearrange("p (t j) -> p t j", j=12),
                                       p4[:, :, :, cw], -0.5, p4[:, :, :, cc],
                                       AL.mult, AL.add)
            for dst, cc, cw in ((px2, 0, 2), (py2, 1, 3)):
                V.scalar_tensor_tensor(dst[:].rearrange("p (t j) -> p t j", j=12),
                                       p4[:, :, :, cw], 0.5, p4[:, :, :, cc],
                                       AL.mult, AL.add)

            # pred-gt IoU
            jx1, jy1, jx2, jy2 = s12(), s12(), s12(), s12()
            tt(jx1[:].rearrange("p (t j) -> p t j", j=12), px1[:].rearrange("p (t j) -> p t j", j=12), GX1, AL.max)
            tt(jy1[:].rearrange("p (t j) -> p t j", j=12), py1[:].rearrange("p (t j) -> p t j", j=12), GY1, AL.max)
            tt(jx2[:].rearrange("p (t j) -> p t j", j=12), px2[:].rearrange("p (t j) -> p t j", j=12), GX2, AL.min)
            tt(jy2[:].rearrange("p (t j) -> p t j", j=12), py2[:].rearrange("p (t j) -> p t j", j=12), GY2, AL.min)
            jw, jh = s12(), s12()
            tt(jw[:], jx2[:], jx1[:], AL.subtract)
            V.tensor_scalar_max(jw[:], jw[:], 0.0)
            tt(jh[:], jy2[:], jy1[:], AL.subtract)
            V.tensor_scalar_max(jh[:], jh[:], 0.0)
            ji = s12()
            tt(ji[:], jw[:], jh[:], AL.mult)
            pa = s12()
            V.tensor_tensor(pa[:].rearrange("p (t j) -> p t j", j=12),
                            p4[:, :, :, 2], p4[:, :, :, 3], AL.mult)
            ju = s12()
            tt(ju[:].rearrange("p (t j) -> p t j", j=12),
               pa[:].rearrange("p (t j) -> p t j", j=12), GAR, AL.add)
            tt(ju[:], ju[:], ji[:], AL.subtract)
            jden = s12()
            V.tensor_scalar_add(jden[:], ju[:], EPS)
            V.reciprocal(jden[:], jden[:])
            jiou = s12()
            tt(jiou[:], ji[:], jden[:], AL.mult)

            # enclosure + giou
            ex1, ey1, ex2, ey2 = s12(), s12(), s12(), s12()
            tt(ex1[:].rearrange("p (t j) -> p t j", j=12), px1[:].rearrange("p (t j) -> p t j", j=12), GX1, AL.min)
            tt(ey1[:].rearrange("p (t j) -> p t j", j=12), py1[:].rearrange("p (t j) -> p t j", j=12), GY1, AL.min)
            tt(ex2[:].rearrange("p (t j) -> p t j", j=12), px2[:].rearrange("p (t j) -> p t j", j=12), GX2, AL.max)
            tt(ey2[:].rearrange("p (t j) -> p t j", j=12), py2[:].rearrange("p (t j) -> p t j", j=12), GY2, AL.max)
            ew, eh = s12(), s12()
            tt(ew[:], ex2[:], ex1[:], AL.subtract)
            tt(eh[:], ey2[:], ey1[:], AL.subtract)
            en = s12()
            tt(en[:], ew[:], eh[:], AL.mult)
            t1 = s12()
            tt(t1[:], en[:], ju[:], AL.subtract)
            t2 = s12()
            V.tensor_scalar_add(t2[:], en[:], EPS)
            V.reciprocal(t2[:], t2[:])
            tt(t1[:], t1[:], t2[:], AL.mult)
            gio = s12()
            tt(gio[:], jiou[:], t1[:], AL.subtract)
            junk2 = s12()
            V.scalar_tensor_tensor(junk2[:], gio[:], 1.0, pos[:], AL.mult, AL.mult,
                                   accum_out=accg[:, 2:3])

            # bce: relu(z) + log1p(exp(-|z|)) - z*pos
            absz = s12()
            V.scalar_tensor_tensor(absz[:].rearrange("p (t j) -> p t j", j=12),
                                   Zv, -1.0, Zv, AL.mult, AL.max)
            ebuf = s12()
            S.activation(ebuf[:], absz[:], AF.Exp, scale=-1.0)
            lbuf = s12()
            S.activation(lbuf[:], ebuf[:], AF.Ln, bias=1.0, accum_out=accg[:, 3:4])
            rbuf = s12()
            S.activation(rbuf[:].rearrange("p (t j) -> p t j", j=12), Zv, AF.Relu,
                         accum_out=accg[:, 4:5])
            junk3 = s12()
            V.scalar_tensor_tensor(junk3[:].rearrange("p (t j) -> p t j", j=12),
                                   Zv, 1.0, pos[:].rearrange("p (t j) -> p t j", j=12),
                                   AL.mult, AL.mult, accum_out=accg[:, 5:6])

            V.tensor_tensor(acc[:], acc[:], accg[:], AL.add)

        fin = cp.tile([1, 6], f32, tag="fin")
        nc.gpsimd.tensor_reduce(fin[:], acc[:], mybir.AxisListType.C, AL.add)
        nc.sync.dma_start(out_d.ap(), fin[:])

    nc.compile()
    return nc


# ---------------------------------------------------------------- host wrapper
def _get_program():
    if "nc" not in _CACHE:
        _CACHE["nc"] = _build_nc()
    return _CACHE["nc"]


def _fold_bn(p):
    s = _np32(p["g"]) / np.sqrt(_np32(p["rv"]) + np.float32(1e-5))
    t = (_np32(p["b"]) - _np32(p["rm"])) * s + _np32(p["bt"])
    return _np32(p["w"]), s.astype(np.float32), t.astype(np.float32)


def _static_inputs(params):
    w0, s0, t0 = _fold_bn(params["in_conv"])
    wr, sr, tr = zip(*[_fold_bn(p) for p in params["reg_convs"]])
    wc, sc, tc_ = zip(*[_fold_bn(p) for p in params["cls_convs"]])
    whr = _np32(params["reg_head"]["w"])       # [48, 256]
    whc = _np32(params["cls_head"]["w"])       # [12, 256]
    bhr = _np32(params["reg_head"]["b"])
    bhc = _np32(params["cls_head"]["b"])

    w0_arr = _np32(w0.T.reshape(2, 128, 512))
    wr_arr = _np32(np.stack([w.T.reshape(2, 128, 256) for w in wr]))
    wc_arr = _np32(np.stack([w.T.reshape(2, 128, 256) for w in wc]))
    wh_arr = np.zeros((4, 128, 60), np.float32)
    wh_arr[0, :, 0:48] = whr.T[0:128]
    wh_arr[1, :, 0:48] = whr.T[128:256]
    wh_arr[2, :, 48:60] = whc.T[0:128]
    wh_arr[3, :, 48:60] = whc.T[128:256]

    sbv = np.zeros((128, 34), np.float32)
    sbv[:, 0:4] = s0.reshape(4, 128).T
    sbv[:, 16:20] = t0.reshape(4, 128).T
    for l in range(3):
        sbv[:, 4 + 2 * l:6 + 2 * l] = sr[l].reshape(2, 128).T
        sbv[:, 20 + 2 * l:22 + 2 * l] = tr[l].reshape(2, 128).T
        sbv[:, 10 + 2 * l:12 + 2 * l] = sc[l].reshape(2, 128).T
        sbv[:, 26 + 2 * l:28 + 2 * l] = tc_[l].reshape(2, 128).T
    sbv[0:48, 32] = bhr
    sbv[48:60, 32] = bhc

    idn = np.eye(128, dtype=np.float32)

    anchors = _make_anchors_np()                       # [3072, 4] xyhw f32
    axy = _xyxy(anchors).astype(np.float32)            # [3072, 4] xyxy
    aar = ((axy[:, 2] - axy[:, 0]) * (axy[:, 3] - axy[:, 1])).astype(np.float32)

    av = anchors.reshape(2, 128, 48)                   # [phase, p, (j c)]
    tph = np.arange(NT) % 2
    anc48 = _np32(av[tph].transpose(1, 0, 2))          # [128, NT, 48]

    axv = axy.reshape(2, 128, 12, 4)
    ancx = np.empty((128, 4, NT, 12), np.float32)
    for k in range(4):
        ancx[:, k] = axv[tph, :, :, k].transpose(1, 0, 2)
    ancx = _np32(ancx)

    aav = aar.reshape(2, 128, 12)                      # [phase, p, j]
    return dict(w0=w0_arr, wr=wr_arr, wc=wc_arr, wh=wh_arr, sbv=sbv, idn=idn,
                anc=anc48, ancx=ancx), aav


def _gt_inputs(gt_core, aav):
    # gt_core [30, 4] xyhw for this core's frames
    gxy = _xyxy(gt_core).astype(np.float32)            # [30, 4]
    gar = ((gxy[:, 2] - gxy[:, 0]) * (gxy[:, 3] - gxy[:, 1])).astype(np.float32)
    tb = np.arange(NT) // 2                            # frame of tile t
    tph = np.arange(NT) % 2

    gt48 = np.broadcast_to(gt_core[tb][None, :, None, :], (128, NT, 12, 4))
    gt48 = _np32(gt48.reshape(128, NT, 48))

    gtx = np.empty((128, 6, NT, 12), np.float32)
    for k in range(4):
        gtx[:, k] = np.broadcast_to(gxy[tb, k][None, :, None], (128, NT, 12))
    gtx[:, 4] = np.broadcast_to(gar[tb][None, :, None], (128, NT, 12))
    # SAB = area_a + area_b in f32 (reference op order: area_a + area_b)
    gtx[:, 5] = aav[tph].transpose(1, 0, 2) + gar[tb][None, :, None]
    return dict(gt48=gt48, gtx=_np32(gtx))


def kernel(feat, gt_boxes, params):
    from concourse.bass_utils import run_bass_kernel_spmd

    feat = _np32(feat)
    gt = _np32(gt_boxes)
    params = {
        k: ([{kk: np.asarray(vv) for kk, vv in p.items()} for p in v]
            if isinstance(v, list) else {kk: np.asarray(vv) for kk, vv in v.items()})
        for k, v in params.items()
    }

    nc = _get_program()
    static, aav = _static_inputs(params)
    in_maps = []
    for ci in range(N_CORES):
        m = dict(static)
        m["feat"] = _np32(feat[BLOC * ci:BLOC * (ci + 1)])
        m.update(_gt_inputs(gt[BLOC * ci:BLOC * (ci + 1)], aav))
        in_maps.append(m)

    res = run_bass_kernel_spmd(nc, in_maps, core_ids=list(range(N_CORES)))
    _CACHE["last_res"] = res
    S = np.zeros(6, np.float64)
    for ci in range(N_CORES):
        S += res.results[ci]["o"][0].astype(np.float64)
    npos_t, s_ch, s_pg, s_ln, s_rl, s_zp = S
    np_clamped = max(npos_t, 1.0)
    loss = (s_ch + 0.3 * (npos_t - s_pg)) / np_clamped \
        + 100.0 * (s_rl + s_ln - s_zp) / (B * 3072)
    return np.float32(loss)
